# revision 28
# baseline (speedup 1.0000x reference)
"""Trainium2 kernel for nn_Policy_36103495090600 (scatter_project + concat).

out[b] = concat(spatial_info[b],                                  # 48 ch
                scatter(zeros[32,256,256], proj[b] at entity_location[b]))

Data-parallel over batch: 16 batches -> 8 cores x 2 batches.

HW indirect-DMA reality (probed): one descriptor per partition of the offset
AP; descriptor p writes the ENTIRE in_ row contiguously at flat element
address idx[p,0]*coef. Per-element (axis=1 multi-index) scatter does not
exist on HW. So we scatter ENTITY-CONTIGUOUS blocks of 32 channels into a
pre-zeroed DRAM stage at flat addr k*2M + hw*32, then reload the stage,
de-interleave [hw,32]->[32,hw] on the vector engine, and write the 64
channel rows of the real output with plain DMAs.

Duplicate (h,w) within a batch: reference (XLA CPU) is last-entity-wins;
host redirects all but the last occurrence into stage garbage rows 256-257
so every descriptor stays in bounds.

Stage relies on the runtime pre-zeroing ExternalOutput buffers.
"""

import numpy as np

from concourse import bacc, bass, mybir, tile
from concourse.bass_utils import run_bass_kernel_spmd
from concourse.masks import make_identity

B, N, D, C = 16, 512, 256, 32
CS, H, WG = 48, 256, 256
HWP = H * WG                  # 65536
NCORES = 8
BPC = B // NCORES             # 2 batches per core
SROW = 16384                  # stage row width (f32)
SROWS_PER_B = C * HWP // SROW  # 128 stage rows per batch
STAGE_ROWS = SROWS_PER_B + 1  # per-batch stage; +1 garbage row for dups
GARB = SROWS_PER_B * SROW     # 2097152: batch-local f32 base of garbage row
NCHUNK = BPC * N // 128       # 8 chunks of 128 entities

_NC = None
LAST_RESULTS = None


def _build():
    f32 = mybir.dt.float32
    # Bacc (not plain Bass): its finalize() runs move_matmul_waits_to_ldweights
    # + generate_event_semaphores, required because walrus codegen allows only
    # one sync wait per instruction.
    nc = bacc.Bacc()
    sp_in = nc.declare_dram_parameter("spatial", [BPC * CS, HWP], f32, isOutput=False)
    emb_in = nc.declare_dram_parameter("emb", [BPC * N, D], f32, isOutput=False)
    w_in = nc.declare_dram_parameter("w", [D, C], f32, isOutput=False)
    b_in = nc.declare_dram_parameter("bias", [1, C], f32, isOutput=False)
    idx_in = nc.declare_dram_parameter(
        "idx", [128, NCHUNK], mybir.dt.int32, isOutput=False
    )
    out_sp = nc.declare_dram_parameter("out_sp", [BPC * CS, HWP], f32, isOutput=True)
    out_sc = nc.declare_dram_parameter("out_sc", [BPC * C, HWP], f32, isOutput=True)
    # one stage tensor per batch: reload(k, h) then only depends on batch
    # k's 4 scatter chunks, so the sync ring starts reloading batch 0 while
    # batch 1's scatters are still draining on the software DGE
    stages = [
        nc.declare_dram_parameter(f"stage{k}", [STAGE_ROWS, SROW], f32, isOutput=True)
        for k in range(BPC)
    ]

    with tile.TileContext(nc) as tc:
        with (
            tc.tile_pool(name="sbuf", bufs=1) as spool,
            tc.tile_pool(name="ebuf", bufs=4) as epool,
            tc.tile_pool(name="big", bufs=3) as bigpool,
            tc.tile_pool(name="big2", bufs=2) as bigpool2,
            tc.tile_pool(name="psum", bufs=2, space="PSUM") as ppool,
        ):
            # All static loads go on the HWDGE rings (sync/scalar) ahead of
            # the bulk spatial traffic: the gpsimd software DGE is starved /
            # descriptor-rate limited (~85 us observed) when the big copies
            # are in flight, which stalls the whole matmul+scatter path.
            ident = spool.tile([128, 128], dtype=f32)
            make_identity(nc, ident[:])
            w_sb = spool.tile([128, 2 * C], dtype=f32)
            nc.sync.dma_start(out=w_sb[:, 0:C], in_=w_in[0:128, :])
            nc.sync.dma_start(out=w_sb[:, C : 2 * C], in_=w_in[128:256, :])
            b_row = spool.tile([1, C], dtype=f32)
            nc.scalar.dma_start(out=b_row[:], in_=b_in[:])
            ones_sb = spool.tile([1, 128], dtype=f32)
            nc.vector.memset(ones_sb[:], 1.0)
            idx_sb = spool.tile([128, NCHUNK], dtype=mybir.dt.int32)
            nc.scalar.dma_start(out=idx_sb[:], in_=idx_in[:])

            # embT[d2][d, r] = emb[r, d2*128+d], r = k*N + n
            embT = [
                spool.tile([128, BPC * N], dtype=f32, name=f"embT{i}")
                for i in range(2)
            ]
            for t in range(NCHUNK):
                e_sb = epool.tile([128, D], dtype=f32)
                eng = nc.sync if t % 2 == 0 else nc.scalar
                eng.dma_start(out=e_sb[:], in_=emb_in[t * 128 : (t + 1) * 128, :])
                for d2 in range(2):
                    ps = ppool.tile([128, 128], dtype=f32, space="PSUM")
                    nc.tensor.transpose(
                        out=ps[:],
                        in_=e_sb[:, d2 * 128 : (d2 + 1) * 128],
                        identity=ident[:],
                    )
                    nc.vector.tensor_copy(
                        out=embT[d2][:, t * 128 : (t + 1) * 128], in_=ps[:]
                    )

            # proj chunk [128 entities, 32 ch] = embT.T @ W + 1s.T @ bias,
            # then one row-descriptor scatter per chunk (128 descriptors,
            # each writes 32 contiguous f32 at flat stage addr idx[p, j])
            for j in range(NCHUNK):
                col = j * 128
                pp = ppool.tile([128, C], dtype=f32, space="PSUM")
                nc.tensor.matmul(
                    out=pp[:],
                    lhsT=embT[0][:, col : col + 128],
                    rhs=w_sb[:, 0:C],
                    start=True,
                    stop=False,
                )
                nc.tensor.matmul(
                    out=pp[:],
                    lhsT=embT[1][:, col : col + 128],
                    rhs=w_sb[:, C : 2 * C],
                    start=False,
                    stop=False,
                )
                nc.tensor.matmul(
                    out=pp[:],
                    lhsT=ones_sb[:],
                    rhs=b_row[:],
                    start=False,
                    stop=True,
                )
                proj_sb = spool.tile([128, C], dtype=f32, name=f"proj{j}")
                nc.vector.tensor_copy(out=proj_sb[:], in_=pp[:])
                nc.gpsimd.indirect_dma_start(
                    out=stages[j // (NCHUNK // BPC)][:],
                    out_offset=bass.IndirectOffsetOnAxis(
                        ap=idx_sb[:, j : j + 1], axis=1
                    ),
                    in_=proj_sb[:],
                    in_offset=None,
                )

            # spatial passthrough: only a small chunk upfront -- the rings
            # must go mostly quiet while the SWDGE scatters drain (heavy
            # HWDGE traffic starves software-DGE descriptor processing).
            # sync carries rows 0:48, scalar 48:96.
            nc.sync.dma_start(out=out_sp[0:10, :], in_=sp_in[0:10, :])
            nc.scalar.dma_start(out=out_sp[48:58, :], in_=sp_in[48:58, :])

            # half-batch pipeline: reload stage half (entity-contiguous),
            # de-interleave [p][q,c] -> [p][c,q] on vector, 32 half-row DMAs.
            # sb_h[p,q,c] = channel c of hw = h*32768 + p*256 + q
            # All 4 reloads enqueued k-major first; batch-0 scatters land
            # first so sync starts reloading while batch 1 still scatters.
            HHALF = HWP // 2
            iters = [(0, 0), (1, 0), (0, 1), (1, 1)]
            sbhs = []
            for k, h in iters:
                eng = nc.sync if k == 0 else nc.scalar
                sb_h = bigpool.tile([128, 256, C], dtype=f32, name="sbh")
                eng.dma_start(out=sb_h[:], in_=stages[k][h * 64 : (h + 1) * 64, :])
                sbhs.append(sb_h)
            starts = [10, 20, 29, 38, 48]
            for i, (k, h) in enumerate(iters):
                s0, s1 = starts[i], starts[i + 1]
                nc.sync.dma_start(out=out_sp[s0:s1, :], in_=sp_in[s0:s1, :])
                nc.scalar.dma_start(
                    out=out_sp[s0 + 48 : s1 + 48, :], in_=sp_in[s0 + 48 : s1 + 48, :]
                )
                sb2_h = bigpool2.tile([128, C, 256], dtype=f32, name="sb2h")
                nc.vector.tensor_copy(
                    out=sb2_h[:], in_=sbhs[i][:].transpose([0, 2, 1])
                )
                for c in range(C):
                    e2 = nc.sync if (c + i) % 2 == 0 else nc.scalar
                    row = k * C + c
                    e2.dma_start(
                        out=out_sc[row : row + 1, h * HHALF : (h + 1) * HHALF],
                        in_=sb2_h[:, c : c + 1, :],
                    )
    nc.finalize()
    return nc


def _get_nc():
    global _NC
    if _NC is None:
        _NC = _build()
    return _NC


def _make_indices(entity_location):
    loc = np.asarray(entity_location).astype(np.int64)
    flat = loc[..., 0] * WG + loc[..., 1]  # [B, N]
    keep = np.ones((B, N), dtype=bool)
    for bi in range(B):
        seen = set()
        fb = flat[bi]
        for n in range(N - 1, -1, -1):
            v = int(fb[n])
            if v in seen:
                keep[bi, n] = False
            else:
                seen.add(v)
    valid = flat * C
    # dups land in the per-batch garbage row (addr >= GARB), each at a
    # unique 32-wide slot so they never clobber live data
    dup = GARB + np.arange(N, dtype=np.int64)[None, :] * C
    idx = np.where(keep, valid, dup)
    return idx.astype(np.int32)  # [B, N] batch-local flat f32 element addrs


def kernel(spatial_info, entity_embeddings, W, b, entity_location):
    global LAST_RESULTS
    spatial = np.ascontiguousarray(spatial_info, dtype=np.float32)
    emb = np.ascontiguousarray(entity_embeddings, dtype=np.float32)
    wm = np.ascontiguousarray(W, dtype=np.float32)
    bv = np.ascontiguousarray(b, dtype=np.float32).reshape(1, C)
    idx_all = _make_indices(entity_location)

    in_maps = []
    for c in range(NCORES):
        b0 = c * BPC
        # idx column j = k*4 + t holds entities t*128..t*128+127 of batch k
        idx_core = (
            idx_all[b0 : b0 + BPC]
            .reshape(BPC, NCHUNK // BPC, 128)
            .transpose(2, 0, 1)
            .reshape(128, NCHUNK)
        )
        in_maps.append(
            {
                "spatial": spatial[b0 : b0 + BPC].reshape(BPC * CS, HWP),
                "emb": emb[b0 : b0 + BPC].reshape(BPC * N, D),
                "w": wm,
                "bias": bv,
                "idx": np.ascontiguousarray(idx_core),
            }
        )

    res = run_bass_kernel_spmd(_get_nc(), in_maps, list(range(NCORES)))
    LAST_RESULTS = res

    out = np.empty((B, CS + C, H, WG), dtype=np.float32)
    for c in range(NCORES):
        b0 = c * BPC
        out[b0 : b0 + BPC, :CS] = res.results[c]["out_sp"].reshape(BPC, CS, H, WG)
        out[b0 : b0 + BPC, CS:] = res.results[c]["out_sc"].reshape(BPC, C, H, WG)
    return out


# revision 30
# speedup vs baseline: 1.0792x; 1.0792x over previous
"""Trainium2 kernel for nn_Policy_36103495090600 (scatter_project + concat).

out[b] = concat(spatial_info[b],                                  # 48 ch
                scatter(zeros[32,256,256], proj[b] at entity_location[b]))

Data-parallel over batch: 16 batches -> 8 cores x 2 batches.

HW indirect-DMA reality (probed): one descriptor per partition of the offset
AP; descriptor p writes the ENTIRE in_ row contiguously at flat element
address idx[p,0]*coef. Per-element (axis=1 multi-index) scatter does not
exist on HW. So we scatter ENTITY-CONTIGUOUS blocks of 32 channels into a
pre-zeroed DRAM stage at flat addr k*2M + hw*32, then reload the stage,
de-interleave [hw,32]->[32,hw] on the vector engine, and write the 64
channel rows of the real output with plain DMAs.

Duplicate (h,w) within a batch: reference (XLA CPU) is last-entity-wins;
host redirects all but the last occurrence into stage garbage rows 256-257
so every descriptor stays in bounds.

Stage relies on the runtime pre-zeroing ExternalOutput buffers.
"""

import numpy as np

from concourse import bacc, bass, mybir, tile
from concourse.bass_utils import run_bass_kernel_spmd
from concourse.masks import make_identity

B, N, D, C = 16, 512, 256, 32
CS, H, WG = 48, 256, 256
HWP = H * WG                  # 65536
NCORES = 8
BPC = B // NCORES             # 2 batches per core
SROW = 16384                  # stage row width (f32)
SROWS_PER_B = C * HWP // SROW  # 128 stage rows per batch
STAGE_ROWS = SROWS_PER_B + 1  # per-batch stage; +1 garbage row for dups
GARB = SROWS_PER_B * SROW     # 2097152: batch-local f32 base of garbage row
NCHUNK = BPC * N // 128       # 8 chunks of 128 entities

_NC = None
LAST_RESULTS = None


def _build():
    f32 = mybir.dt.float32
    # Bacc (not plain Bass): its finalize() runs move_matmul_waits_to_ldweights
    # + generate_event_semaphores, required because walrus codegen allows only
    # one sync wait per instruction.
    nc = bacc.Bacc()
    sp_in = nc.declare_dram_parameter("spatial", [BPC * CS, HWP], f32, isOutput=False)
    emb_in = nc.declare_dram_parameter("emb", [BPC * N, D], f32, isOutput=False)
    w_in = nc.declare_dram_parameter("w", [D, C], f32, isOutput=False)
    b_in = nc.declare_dram_parameter("bias", [1, C], f32, isOutput=False)
    idx_in = nc.declare_dram_parameter(
        "idx", [128, NCHUNK], mybir.dt.int32, isOutput=False
    )
    out_sp = nc.declare_dram_parameter("out_sp", [BPC * CS, HWP], f32, isOutput=True)
    out_sc = nc.declare_dram_parameter("out_sc", [BPC * C, HWP], f32, isOutput=True)
    # one stage tensor per batch: reload(k, h) then only depends on batch
    # k's 4 scatter chunks, so the sync ring starts reloading batch 0 while
    # batch 1's scatters are still draining on the software DGE
    stages = [
        nc.declare_dram_parameter(f"stage{k}", [STAGE_ROWS, SROW], f32, isOutput=True)
        for k in range(BPC)
    ]

    with tile.TileContext(nc) as tc:
        with (
            tc.tile_pool(name="sbuf", bufs=1) as spool,
            tc.tile_pool(name="ebuf", bufs=4) as epool,
            tc.tile_pool(name="big", bufs=3) as bigpool,
            tc.tile_pool(name="big2", bufs=2) as bigpool2,
            tc.tile_pool(name="psum", bufs=2, space="PSUM") as ppool,
        ):
            # All static loads go on the HWDGE rings (sync/scalar) ahead of
            # the bulk spatial traffic: the gpsimd software DGE is starved /
            # descriptor-rate limited (~85 us observed) when the big copies
            # are in flight, which stalls the whole matmul+scatter path.
            ident = spool.tile([128, 128], dtype=f32)
            make_identity(nc, ident[:])
            w_sb = spool.tile([128, 2 * C], dtype=f32)
            nc.sync.dma_start(out=w_sb[:, 0:C], in_=w_in[0:128, :])
            nc.sync.dma_start(out=w_sb[:, C : 2 * C], in_=w_in[128:256, :])
            b_row = spool.tile([1, C], dtype=f32)
            nc.scalar.dma_start(out=b_row[:], in_=b_in[:])
            ones_sb = spool.tile([1, 128], dtype=f32)
            nc.vector.memset(ones_sb[:], 1.0)
            idx_sb = spool.tile([128, NCHUNK], dtype=mybir.dt.int32)
            nc.scalar.dma_start(out=idx_sb[:], in_=idx_in[:])

            # embT[d2][d, r] = emb[r, d2*128+d], r = k*N + n
            embT = [
                spool.tile([128, BPC * N], dtype=f32, name=f"embT{i}")
                for i in range(2)
            ]
            for t in range(NCHUNK):
                e_sb = epool.tile([128, D], dtype=f32)
                eng = nc.sync if t % 2 == 0 else nc.scalar
                eng.dma_start(out=e_sb[:], in_=emb_in[t * 128 : (t + 1) * 128, :])
                for d2 in range(2):
                    ps = ppool.tile([128, 128], dtype=f32, space="PSUM")
                    nc.tensor.transpose(
                        out=ps[:],
                        in_=e_sb[:, d2 * 128 : (d2 + 1) * 128],
                        identity=ident[:],
                    )
                    nc.vector.tensor_copy(
                        out=embT[d2][:, t * 128 : (t + 1) * 128], in_=ps[:]
                    )

            # proj chunk [128 entities, 32 ch] = embT.T @ W + 1s.T @ bias,
            # then one row-descriptor scatter per chunk (128 descriptors,
            # each writes 32 contiguous f32 at flat stage addr idx[p, j])
            for j in range(NCHUNK):
                col = j * 128
                pp = ppool.tile([128, C], dtype=f32, space="PSUM")
                nc.tensor.matmul(
                    out=pp[:],
                    lhsT=embT[0][:, col : col + 128],
                    rhs=w_sb[:, 0:C],
                    start=True,
                    stop=False,
                )
                nc.tensor.matmul(
                    out=pp[:],
                    lhsT=embT[1][:, col : col + 128],
                    rhs=w_sb[:, C : 2 * C],
                    start=False,
                    stop=False,
                )
                nc.tensor.matmul(
                    out=pp[:],
                    lhsT=ones_sb[:],
                    rhs=b_row[:],
                    start=False,
                    stop=True,
                )
                proj_sb = spool.tile([128, C], dtype=f32, name=f"proj{j}")
                nc.vector.tensor_copy(out=proj_sb[:], in_=pp[:])
                nc.gpsimd.indirect_dma_start(
                    out=stages[j // (NCHUNK // BPC)][:],
                    out_offset=bass.IndirectOffsetOnAxis(
                        ap=idx_sb[:, j : j + 1], axis=1
                    ),
                    in_=proj_sb[:],
                    in_offset=None,
                )

            # spatial passthrough: bulk upfront keeps the HWDGE rings busy
            # while the SWDGE scatters dribble out (heavy HWDGE traffic
            # starves software-DGE descriptor processing either way; the
            # scheduler floods all ready DMAs, so give it the bulk here).
            # sync carries rows 0:48, scalar 48:96.
            nc.sync.dma_start(out=out_sp[0:36, :], in_=sp_in[0:36, :])
            nc.scalar.dma_start(out=out_sp[48:84, :], in_=sp_in[48:84, :])

            # half-batch pipeline: reload stage half (entity-contiguous),
            # de-interleave [p][q,c] -> [p][c,q] on vector, 32 half-row DMAs.
            # sb_h[p,q,c] = channel c of hw = h*32768 + p*256 + q
            # All 4 reloads enqueued k-major first; batch-0 scatters land
            # first so sync starts reloading while batch 1 still scatters.
            HHALF = HWP // 2
            iters = [(0, 0), (1, 0), (0, 1), (1, 1)]
            sbhs = []
            for k, h in iters:
                eng = nc.sync if k == 0 else nc.scalar
                sb_h = bigpool.tile([128, 256, C], dtype=f32, name="sbh")
                eng.dma_start(out=sb_h[:], in_=stages[k][h * 64 : (h + 1) * 64, :])
                sbhs.append(sb_h)
            starts = [36, 39, 42, 45, 48]
            for i, (k, h) in enumerate(iters):
                s0, s1 = starts[i], starts[i + 1]
                nc.sync.dma_start(out=out_sp[s0:s1, :], in_=sp_in[s0:s1, :])
                nc.scalar.dma_start(
                    out=out_sp[s0 + 48 : s1 + 48, :], in_=sp_in[s0 + 48 : s1 + 48, :]
                )
                sb2_h = bigpool2.tile([128, C, 256], dtype=f32, name="sb2h")
                nc.vector.tensor_copy(
                    out=sb2_h[:], in_=sbhs[i][:].transpose([0, 2, 1])
                )
                for c in range(C):
                    e2 = nc.sync if (c + i) % 2 == 0 else nc.scalar
                    row = k * C + c
                    e2.dma_start(
                        out=out_sc[row : row + 1, h * HHALF : (h + 1) * HHALF],
                        in_=sb2_h[:, c : c + 1, :],
                    )
    nc.finalize()
    return nc


def _get_nc():
    global _NC
    if _NC is None:
        _NC = _build()
    return _NC


def _make_indices(entity_location):
    loc = np.asarray(entity_location).astype(np.int64)
    flat = loc[..., 0] * WG + loc[..., 1]  # [B, N]
    keep = np.ones((B, N), dtype=bool)
    for bi in range(B):
        seen = set()
        fb = flat[bi]
        for n in range(N - 1, -1, -1):
            v = int(fb[n])
            if v in seen:
                keep[bi, n] = False
            else:
                seen.add(v)
    valid = flat * C
    # dups land in the per-batch garbage row (addr >= GARB), each at a
    # unique 32-wide slot so they never clobber live data
    dup = GARB + np.arange(N, dtype=np.int64)[None, :] * C
    idx = np.where(keep, valid, dup)
    return idx.astype(np.int32)  # [B, N] batch-local flat f32 element addrs


def kernel(spatial_info, entity_embeddings, W, b, entity_location):
    global LAST_RESULTS
    spatial = np.ascontiguousarray(spatial_info, dtype=np.float32)
    emb = np.ascontiguousarray(entity_embeddings, dtype=np.float32)
    wm = np.ascontiguousarray(W, dtype=np.float32)
    bv = np.ascontiguousarray(b, dtype=np.float32).reshape(1, C)
    idx_all = _make_indices(entity_location)

    in_maps = []
    for c in range(NCORES):
        b0 = c * BPC
        # idx column j = k*4 + t holds entities t*128..t*128+127 of batch k
        idx_core = (
            idx_all[b0 : b0 + BPC]
            .reshape(BPC, NCHUNK // BPC, 128)
            .transpose(2, 0, 1)
            .reshape(128, NCHUNK)
        )
        in_maps.append(
            {
                "spatial": spatial[b0 : b0 + BPC].reshape(BPC * CS, HWP),
                "emb": emb[b0 : b0 + BPC].reshape(BPC * N, D),
                "w": wm,
                "bias": bv,
                "idx": np.ascontiguousarray(idx_core),
            }
        )

    res = run_bass_kernel_spmd(_get_nc(), in_maps, list(range(NCORES)))
    LAST_RESULTS = res

    out = np.empty((B, CS + C, H, WG), dtype=np.float32)
    for c in range(NCORES):
        b0 = c * BPC
        out[b0 : b0 + BPC, :CS] = res.results[c]["out_sp"].reshape(BPC, CS, H, WG)
        out[b0 : b0 + BPC, CS:] = res.results[c]["out_sc"].reshape(BPC, C, H, WG)
    return out


# revision 31
# speedup vs baseline: 1.1331x; 1.0499x over previous
"""Trainium2 kernel for nn_Policy_36103495090600 (scatter_project + concat).

out[b] = concat(spatial_info[b],                                  # 48 ch
                scatter(zeros[32,256,256], proj[b] at entity_location[b]))

Data-parallel over batch: 16 batches -> 8 cores x 2 batches.

HW indirect-DMA reality (probed): one descriptor per partition of the offset
AP; descriptor p writes the ENTIRE in_ row contiguously at flat element
address idx[p,0]*coef. Per-element (axis=1 multi-index) scatter does not
exist on HW. So we scatter ENTITY-CONTIGUOUS blocks of 32 channels into a
pre-zeroed DRAM stage at flat addr k*2M + hw*32, then reload the stage,
de-interleave [hw,32]->[32,hw] on the vector engine, and write the 64
channel rows of the real output with plain DMAs.

Duplicate (h,w) within a batch: reference (XLA CPU) is last-entity-wins;
host redirects all but the last occurrence into stage garbage rows 256-257
so every descriptor stays in bounds.

Stage relies on the runtime pre-zeroing ExternalOutput buffers.
"""

import numpy as np

from concourse import bacc, bass, mybir, tile
from concourse.bass_utils import run_bass_kernel_spmd
from concourse.masks import make_identity

B, N, D, C = 16, 512, 256, 32
CS, H, WG = 48, 256, 256
HWP = H * WG                  # 65536
NCORES = 8
BPC = B // NCORES             # 2 batches per core
SROW = 16384                  # stage row width (f32)
SROWS_PER_B = C * HWP // SROW  # 128 stage rows per batch
STAGE_ROWS = SROWS_PER_B + 1  # per-batch stage; +1 garbage row for dups
GARB = SROWS_PER_B * SROW     # 2097152: batch-local f32 base of garbage row
NCHUNK = BPC * N // 128       # 8 chunks of 128 entities

_NC = None
LAST_RESULTS = None


def _build():
    f32 = mybir.dt.float32
    # Bacc (not plain Bass): its finalize() runs move_matmul_waits_to_ldweights
    # + generate_event_semaphores, required because walrus codegen allows only
    # one sync wait per instruction.
    nc = bacc.Bacc()
    sp_in = nc.declare_dram_parameter("spatial", [BPC * CS, HWP], f32, isOutput=False)
    emb_in = nc.declare_dram_parameter("emb", [BPC * N, D], f32, isOutput=False)
    w_in = nc.declare_dram_parameter("w", [D, C], f32, isOutput=False)
    b_in = nc.declare_dram_parameter("bias", [1, C], f32, isOutput=False)
    idx_in = nc.declare_dram_parameter(
        "idx", [128, NCHUNK], mybir.dt.int32, isOutput=False
    )
    out_sp = nc.declare_dram_parameter("out_sp", [BPC * CS, HWP], f32, isOutput=True)
    out_sc = nc.declare_dram_parameter("out_sc", [BPC * C, HWP], f32, isOutput=True)
    # one stage tensor per batch: reload(k, h) then only depends on batch
    # k's 4 scatter chunks, so the sync ring starts reloading batch 0 while
    # batch 1's scatters are still draining on the software DGE
    stages = [
        nc.declare_dram_parameter(f"stage{k}", [STAGE_ROWS, SROW], f32, isOutput=True)
        for k in range(BPC)
    ]

    with tile.TileContext(nc) as tc:
        with (
            tc.tile_pool(name="sbuf", bufs=1) as spool,
            tc.tile_pool(name="ebuf", bufs=4) as epool,
            tc.tile_pool(name="big", bufs=3) as bigpool,
            tc.tile_pool(name="big2", bufs=2) as bigpool2,
            tc.tile_pool(name="psum", bufs=2, space="PSUM") as ppool,
        ):
            # All static loads go on the HWDGE rings (sync/scalar) ahead of
            # the bulk spatial traffic: the gpsimd software DGE is starved /
            # descriptor-rate limited (~85 us observed) when the big copies
            # are in flight, which stalls the whole matmul+scatter path.
            ident = spool.tile([128, 128], dtype=f32)
            make_identity(nc, ident[:])
            w_sb = spool.tile([128, 2 * C], dtype=f32)
            nc.sync.dma_start(out=w_sb[:, 0:C], in_=w_in[0:128, :])
            nc.sync.dma_start(out=w_sb[:, C : 2 * C], in_=w_in[128:256, :])
            b_row = spool.tile([1, C], dtype=f32)
            nc.scalar.dma_start(out=b_row[:], in_=b_in[:])
            ones_sb = spool.tile([1, 128], dtype=f32)
            nc.vector.memset(ones_sb[:], 1.0)
            idx_sb = spool.tile([128, NCHUNK], dtype=mybir.dt.int32)
            nc.scalar.dma_start(out=idx_sb[:], in_=idx_in[:])

            # embT[d2][d, r] = emb[r, d2*128+d], r = k*N + n
            embT = [
                spool.tile([128, BPC * N], dtype=f32, name=f"embT{i}")
                for i in range(2)
            ]
            for t in range(NCHUNK):
                e_sb = epool.tile([128, D], dtype=f32)
                eng = nc.sync if t % 2 == 0 else nc.scalar
                eng.dma_start(out=e_sb[:], in_=emb_in[t * 128 : (t + 1) * 128, :])
                for d2 in range(2):
                    ps = ppool.tile([128, 128], dtype=f32, space="PSUM")
                    nc.tensor.transpose(
                        out=ps[:],
                        in_=e_sb[:, d2 * 128 : (d2 + 1) * 128],
                        identity=ident[:],
                    )
                    nc.vector.tensor_copy(
                        out=embT[d2][:, t * 128 : (t + 1) * 128], in_=ps[:]
                    )

            # proj chunk [128 entities, 32 ch] = embT.T @ W + 1s.T @ bias,
            # then one row-descriptor scatter per chunk (128 descriptors,
            # each writes 32 contiguous f32 at flat stage addr idx[p, j])
            for j in range(NCHUNK):
                col = j * 128
                pp = ppool.tile([128, C], dtype=f32, space="PSUM")
                nc.tensor.matmul(
                    out=pp[:],
                    lhsT=embT[0][:, col : col + 128],
                    rhs=w_sb[:, 0:C],
                    start=True,
                    stop=False,
                )
                nc.tensor.matmul(
                    out=pp[:],
                    lhsT=embT[1][:, col : col + 128],
                    rhs=w_sb[:, C : 2 * C],
                    start=False,
                    stop=False,
                )
                nc.tensor.matmul(
                    out=pp[:],
                    lhsT=ones_sb[:],
                    rhs=b_row[:],
                    start=False,
                    stop=True,
                )
                proj_sb = spool.tile([128, C], dtype=f32, name=f"proj{j}")
                nc.vector.tensor_copy(out=proj_sb[:], in_=pp[:])
                nc.gpsimd.indirect_dma_start(
                    out=stages[j // (NCHUNK // BPC)][:],
                    out_offset=bass.IndirectOffsetOnAxis(
                        ap=idx_sb[:, j : j + 1], axis=1
                    ),
                    in_=proj_sb[:],
                    in_offset=None,
                )

            # spatial passthrough: bulk upfront keeps the HWDGE rings busy
            # while the SWDGE scatters dribble out (heavy HWDGE traffic
            # starves software-DGE descriptor processing either way; the
            # scheduler floods all ready DMAs, so give it the bulk here).
            # sync carries rows 0:48, scalar 48:96.
            # issued as 3-row pieces: inter-DMA boundaries give the SWDGE
            # periodic arbitration windows so scatters drain sooner
            for s in range(0, 36, 3):
                nc.sync.dma_start(out=out_sp[s : s + 3, :], in_=sp_in[s : s + 3, :])
                nc.scalar.dma_start(
                    out=out_sp[s + 48 : s + 51, :], in_=sp_in[s + 48 : s + 51, :]
                )

            # half-batch pipeline: reload stage half (entity-contiguous),
            # de-interleave [p][q,c] -> [p][c,q] on vector, 32 half-row DMAs.
            # sb_h[p,q,c] = channel c of hw = h*32768 + p*256 + q
            # All 4 reloads enqueued k-major first; batch-0 scatters land
            # first so sync starts reloading while batch 1 still scatters.
            HHALF = HWP // 2
            iters = [(0, 0), (1, 0), (0, 1), (1, 1)]
            sbhs = []
            for k, h in iters:
                eng = nc.sync if k == 0 else nc.scalar
                sb_h = bigpool.tile([128, 256, C], dtype=f32, name="sbh")
                eng.dma_start(out=sb_h[:], in_=stages[k][h * 64 : (h + 1) * 64, :])
                sbhs.append(sb_h)
            starts = [36, 39, 42, 45, 48]
            for i, (k, h) in enumerate(iters):
                s0, s1 = starts[i], starts[i + 1]
                nc.sync.dma_start(out=out_sp[s0:s1, :], in_=sp_in[s0:s1, :])
                nc.scalar.dma_start(
                    out=out_sp[s0 + 48 : s1 + 48, :], in_=sp_in[s0 + 48 : s1 + 48, :]
                )
                sb2_h = bigpool2.tile([128, C, 256], dtype=f32, name="sb2h")
                nc.vector.tensor_copy(
                    out=sb2_h[:], in_=sbhs[i][:].transpose([0, 2, 1])
                )
                for c in range(C):
                    e2 = nc.sync if (c + i) % 2 == 0 else nc.scalar
                    row = k * C + c
                    e2.dma_start(
                        out=out_sc[row : row + 1, h * HHALF : (h + 1) * HHALF],
                        in_=sb2_h[:, c : c + 1, :],
                    )
    nc.finalize()
    return nc


def _get_nc():
    global _NC
    if _NC is None:
        _NC = _build()
    return _NC


def _make_indices(entity_location):
    loc = np.asarray(entity_location).astype(np.int64)
    flat = loc[..., 0] * WG + loc[..., 1]  # [B, N]
    keep = np.ones((B, N), dtype=bool)
    for bi in range(B):
        seen = set()
        fb = flat[bi]
        for n in range(N - 1, -1, -1):
            v = int(fb[n])
            if v in seen:
                keep[bi, n] = False
            else:
                seen.add(v)
    valid = flat * C
    # dups land in the per-batch garbage row (addr >= GARB), each at a
    # unique 32-wide slot so they never clobber live data
    dup = GARB + np.arange(N, dtype=np.int64)[None, :] * C
    idx = np.where(keep, valid, dup)
    return idx.astype(np.int32)  # [B, N] batch-local flat f32 element addrs


def kernel(spatial_info, entity_embeddings, W, b, entity_location):
    global LAST_RESULTS
    spatial = np.ascontiguousarray(spatial_info, dtype=np.float32)
    emb = np.ascontiguousarray(entity_embeddings, dtype=np.float32)
    wm = np.ascontiguousarray(W, dtype=np.float32)
    bv = np.ascontiguousarray(b, dtype=np.float32).reshape(1, C)
    idx_all = _make_indices(entity_location)

    in_maps = []
    for c in range(NCORES):
        b0 = c * BPC
        # idx column j = k*4 + t holds entities t*128..t*128+127 of batch k
        idx_core = (
            idx_all[b0 : b0 + BPC]
            .reshape(BPC, NCHUNK // BPC, 128)
            .transpose(2, 0, 1)
            .reshape(128, NCHUNK)
        )
        in_maps.append(
            {
                "spatial": spatial[b0 : b0 + BPC].reshape(BPC * CS, HWP),
                "emb": emb[b0 : b0 + BPC].reshape(BPC * N, D),
                "w": wm,
                "bias": bv,
                "idx": np.ascontiguousarray(idx_core),
            }
        )

    res = run_bass_kernel_spmd(_get_nc(), in_maps, list(range(NCORES)))
    LAST_RESULTS = res

    out = np.empty((B, CS + C, H, WG), dtype=np.float32)
    for c in range(NCORES):
        b0 = c * BPC
        out[b0 : b0 + BPC, :CS] = res.results[c]["out_sp"].reshape(BPC, CS, H, WG)
        out[b0 : b0 + BPC, CS:] = res.results[c]["out_sc"].reshape(BPC, C, H, WG)
    return out
